# revision 17
# baseline (speedup 1.0000x reference)
"""Multi-head attention (B=2, S=2048, H=1024, NH=16) on 8 TRN2 NeuronCores.

Sharding: fully data/tensor parallel, no collectives. Core c = (b, hg) with
b = c // 4 (batch), hg = c % 4 (head group of 4 heads = 256 of the 1024
projection output dims). Each core projects q/k/v for its 4 heads and runs
flash-style attention fully on-chip.

v4 design (from v3 trace analysis: PE busy 161us was the wall, ACT 131us):
  - 16 warmup matmuls on a memset tile at t=0 release the HAM clock gate
    (PE runs at 1.2GHz until ~3.4us of sustained activity) while input DMAs
    land, so projections start at 2.4GHz.
  - q/k projections use 1024-wide moving operands (2 PSUM-bank pp tiles,
    2 matmuls per contraction chunk) - halves matmul count vs v3, same
    cycles, less per-instruction latency exposure.
  - v projection emits v^T directly (stationary = x chunk, moving = Wv), so
    the 32 PE transposes of v3 are gone and va65 is built straight from the
    projection PSUM by one strided DVE copy per S-chunk.
  - ctx: ONE matmul per (sweep, kc) with full 128-row contraction and a
    [128, 65] va stationary (64 v dims + em column -> row 64 = softmax
    denominator). v3's X/Y row-split was pointless: same cycles, but 2x the
    PSUM banks, 2x the matmuls and an extra DVE add per sweep. Single
    [65, 512] fp32 accumulator per sweep = 1 PSUM bank, copied once to SBUF
    and DMA'd out as fp32 (host divides + transposes; fp32 out removes the
    bf16 rounding of num/den of v3).
  - ScalarE does nothing but exp: A2 groups ([128,2048], 4 banks) and B1
    groups ([128,1024], 2 banks). NDVE of the B1 groups can be moved to the
    Vector engine via a one-instruction Schraudolph exp (tensor_scalar
    fp32->int16 whose integer result IS the bf16 bit pattern of exp), to
    rebalance when ACT is the wall. Default 0.
  - PSUM budget: phase P = pp0+pp1 (4 banks) + A2 inject region (4);
    v phase = pv (1) + A2 (4) + B1 (2); phase 2 = A2(4)+B1(2)+ctx(2x1).
"""

import functools
import sys

if "/opt/trn_rl_repo" not in sys.path:
    sys.path.insert(0, "/opt/trn_rl_repo")

import numpy as np

B, S, H = 2, 2048, 1024
NH, HD = 16, 64
NCORES = 8
GROUPS = 4                # head groups (cores per batch)
DPG = H // GROUPS         # projection dims per core = 256
HPG = DPG // HD           # heads per core = 4
P = 128                   # SBUF partitions
NHC = H // P              # contraction chunks per projection = 8
QB = 512                  # q block
NQB = S // QB             # 4
NKC = S // P              # k chunks = 16
VA_W = HD + 1             # va block width per (kc, h) = 65
OU_H = HD + 1             # 65 output rows per sweep (64 dims + denom)
NJ = NQB * 2              # 8 (qc, hp) sweeps
NU = NJ * NKC             # 128 scores units
NDVE = 16                 # B1 units exp'd on VectorE (Schraudolph)

# Schraudolph exp for the DVE path: int16(round(s*EXPA + EXPB)) bit-viewed
# as bf16 equals exp(0.125*s) within ~3.3% (C=5.5 centers the sawtooth).
EXPA = 0.125 * 128.0 / float(np.log(2.0))
EXPB = 127.0 * 128.0 - 5.5


@functools.lru_cache(maxsize=2)
def _build(bias_zero=True, mask_zero=True, ndve=NDVE):
    import concourse.bacc as bacc
    import concourse.mybir as mybir
    import concourse.tile as tile

    F32 = mybir.dt.float32
    BF16 = mybir.dt.bfloat16
    I16 = mybir.dt.int16
    Exp = mybir.ActivationFunctionType.Exp
    MULT = mybir.AluOpType.mult
    ADD = mybir.AluOpType.add

    nc = bacc.Bacc()

    # x tensors pre-tiled on host: row (pr*2+hf)*128+p, col h*1024+c holds
    # xT[hf*512+h*128+p, pr*1024+c] -> each x_load is a contiguous 2D DMA
    # (8KB per partition line, minimal descriptor count).
    xq_d = nc.declare_dram_parameter("xq", [4 * P, 4096], BF16, isOutput=False)
    xk_d = nc.declare_dram_parameter("xk", [4 * P, 4096], BF16, isOutput=False)
    xv_d = nc.declare_dram_parameter("xv", [4 * P, 4096], BF16, isOutput=False)
    wq_d = nc.declare_dram_parameter("wq", [P, NHC * DPG], BF16, isOutput=False)
    wk_d = nc.declare_dram_parameter("wk", [P, NHC * DPG], BF16, isOutput=False)
    wv_d = nc.declare_dram_parameter("wv", [P, NHC * DPG], BF16, isOutput=False)
    bqk_d = nc.declare_dram_parameter("bqk", [P, 4], F32, isOutput=False)
    bvt_d = nc.declare_dram_parameter("bvt", [P, DPG], F32, isOutput=False)
    em_d = nc.declare_dram_parameter("em", [P, NKC], F32, isOutput=False)
    out_d = nc.declare_dram_parameter("out", [HPG * OU_H, S], F32, isOutput=True)

    # stream order: j = qc*2 + hp. Segmented by projection readiness:
    # seg1 (j0-3, kc0-7) needs only kT(kc0-7)+qT(qc0,1).
    units = (
        [(j, kc) for j in range(4) for kc in range(8)]
        + [(j, kc) for j in range(4) for kc in range(8, 16)]
        + [(j, kc) for j in range(4, NJ) for kc in range(NKC)]
    )
    pos_of = {jk: i for i, jk in enumerate(units)}
    assert len(units) == NU

    PA_BUFS = 12
    PB_BUFS = 24

    with tile.TileContext(nc) as tc:
        with (
            tc.tile_pool(name="const", bufs=1) as cpool,
            tc.tile_pool(name="proj_out", bufs=1) as projpool,
            tc.tile_pool(name="xt", bufs=6) as xpool,
            tc.tile_pool(name="pA", bufs=PA_BUFS) as ppA,
            tc.tile_pool(name="pB", bufs=PB_BUFS) as ppB,
            tc.tile_pool(name="ou", bufs=4) as oupool,
            tc.tile_pool(name="psA", bufs=1, space="PSUM") as psA,
        ):
            # ---- warmup: release the PE HAM clock gate while DMAs land,
            # and preload the exp ACT table so the first real exp is cheap ----
            warm_sb = cpool.tile([P, DPG], BF16)
            nc.gpsimd.memset(warm_sb[:], 0.0)
            warm_e = cpool.tile([P, 1], BF16)
            nc.scalar.activation(warm_e[:], warm_sb[:, 0:1], Exp)
            with tc.tile_pool(name="psW", bufs=1, space="PSUM") as psW:
                wps = psW.tile([P, DPG], F32, tag="warm", name="warm")
                for _ in range(28):
                    nc.tensor.matmul(
                        wps[:], warm_sb[:, 0:P], warm_sb[:], start=True, stop=True
                    )

            # ---- constants; split w loads so proj hc0 can start as soon as
            # the first half of wk + xk lands (DMA is ~0.35GB/s serial) ----
            wq_sb = cpool.tile([P, NHC * DPG], BF16)
            wk_sb = cpool.tile([P, NHC * DPG], BF16)
            wv_sb = cpool.tile([P, NHC * DPG], BF16)

            def w_load(w_sb, w_d, hf=None):
                if hf is None:
                    nc.sync.dma_start(w_sb[:], w_d[:])
                    return
                nc.sync.dma_start(
                    w_sb[:, hf * 1024: (hf + 1) * 1024],
                    w_d[:, hf * 1024: (hf + 1) * 1024],
                )

            w_load(wk_sb, wk_d, 0)

            # ---- persistent projection outputs ----
            qT0 = projpool.tile([P, S], BF16)   # heads 0,1 (hp=0)
            qT1 = projpool.tile([P, S], BF16)   # heads 2,3 (hp=1)
            kT0 = projpool.tile([P, S], BF16)
            kT1 = projpool.tile([P, S], BF16)
            va65 = projpool.tile([P, NKC * HPG * VA_W], BF16)
            nc.gpsimd.memset(va65[:], 0.0)

            qT = (qT0, qT1)
            kT = (kT0, kT1)

            # ---- scores/exp stream machinery ----
            pq = [None] * NU          # (j*NKC+kc) -> (p_tile, col offset)
            state = {"u": 0, "ready": 0, "a2inj": 0, "b1inj": 0, "injc": 0,
                     "b1n": 0}
            cons = [0] * NU           # ctx slots emitted per unit (2 = done)
            from collections import deque
            ring_hist = {"pA": deque(), "pB": deque(), "pA1": deque()}
            ring_bufs = {"pA": PA_BUFS, "pB": PB_BUFS, "pA1": 2}
            hooks = {"force_drain": None}

            def ensure_ring(tag, new_units):
                # before reusing a ring slot, its previous tile's ctx
                # consumers must have been emitted (else silent corruption)
                dq = ring_hist[tag]
                if len(dq) >= ring_bufs[tag]:
                    old = dq.popleft()
                    while any(cons[p] < 2 for p in old):
                        assert hooks["force_drain"] is not None, \
                            "ring wrap before ctx drain exists"
                        hooks["force_drain"]()
                dq.append(new_units)

            def scores_group(kind, psum_pool):
                n = 2 if kind == "A2" else 1
                u0 = state["u"]
                fd = 1024 * n
                if kind == "B1":
                    s = psum_pool.tile([P, 1024], F32, tag="sB", name="sB")
                else:
                    s = psum_pool.tile([P, 2048], F32, tag="sA", name="sA")
                for i in range(n):
                    j, kc = units[u0 + i]
                    qc, hp = divmod(j, 2)
                    for half in range(2):
                        rows = slice(half * 64, half * 64 + 64)
                        nc.tensor.matmul(
                            s[:, i * 1024 + half * 512: i * 1024 + (half + 1) * 512],
                            kT[hp][rows, kc * P: (kc + 1) * P],
                            qT[hp][rows, qc * QB: (qc + 1) * QB],
                            start=True,
                            stop=True,
                        )
                # p-tile allocation after the MMs: the ring guard's forced
                # ctx drains then cannot delay this group's score matmuls
                ensure_ring(
                    {"B1": "pB", "A2": "pA", "A1": "pA1"}[kind],
                    [u0 + i for i in range(n)],
                )
                if kind == "B1":
                    pt = ppB.tile([P, 1024], BF16, tag="pB", name="pB")
                elif kind == "A2":
                    pt = ppA.tile([P, 2048], BF16, tag="pA", name="pA")
                else:
                    pt = ppA.tile([P, 1024], BF16, tag="pA1", name="pA1", bufs=2)
                # B1 exp offload to VectorE: only in the main-loop era (the
                # injected groups must stay on ScalarE to cover its head, and
                # early DVE-exp bursts would starve the va65 build), and
                # alternating so the DVE also keeps up with ctx evacs.
                state["b1c"] = state.get("b1c", 0) + 1
                if (kind == "B1" and state["b1n"] < ndve and u0 >= 64
                        and state["b1c"] % 2 == 0):
                    state["b1n"] += 1
                    nc.vector.tensor_scalar(
                        pt[:, :fd].bitcast(I16), s[:, :fd], EXPA, EXPB, MULT, ADD
                    )
                else:
                    nc.scalar.activation(pt[:, :fd], s[:, :fd], Exp, scale=0.125)
                for i in range(n):
                    j, kc = units[u0 + i]
                    pq[j * NKC + kc] = (pt, i * 1024)
                state["u"] = u0 + n

            # ---- projection machinery ----
            def x_load_half(x_d, pr, hf):
                # half a pair (4 H-chunks x 1024 q-cols): contiguous 2D DMA
                xt = xpool.tile([P, 4096], BF16, tag="xt")
                r0 = (pr * 2 + hf) * P
                nc.sync.dma_start(xt[:], x_d[r0: r0 + P, :])
                return xt

            def inject():
                state["injc"] += 1
                if state["injc"] % 3 == 1:
                    return
                rem = state["ready"] - state["u"]
                if rem >= 2 and state["a2inj"] < 10:
                    state["a2inj"] += 1
                    scores_group("A2", psA)
                elif rem >= 1 and state["b1inj"] < 16 and psB_box[0] is not None:
                    state["b1inj"] += 1
                    scores_group("B1", psB_box[0])

            def proj_pair(halves, w_sb, bcol, dst0, dst1, pr, psP, inj,
                          hook0=None, hook4=None):
                # matmul output must fit one PSUM bank -> 4 x [128, 512]
                pp = [
                    psP.tile([P, QB], F32, tag=f"pp{i}", name=f"pp{i}", bufs=1)
                    for i in range(4)
                ]
                for hc in range(NHC):
                    if hc == 0 and hook0 is not None:
                        hook0()
                    if hc == 4 and hook4 is not None:
                        hook4()
                    if inj:
                        inject()
                    st = dict(start=(hc == 0), stop=(hc == NHC - 1))
                    xh = halves[hc // 4]
                    base = (hc % 4) * 1024
                    xs0 = xh[:, base: base + QB]
                    xs1 = xh[:, base + QB: base + 1024]
                    w0 = w_sb[:, hc * DPG: hc * DPG + P]
                    w1 = w_sb[:, hc * DPG + P: (hc + 1) * DPG]
                    nc.tensor.matmul(pp[0][:], w0, xs0, **st)
                    nc.tensor.matmul(pp[1][:], w0, xs1, **st)
                    nc.tensor.matmul(pp[2][:], w1, xs0, **st)
                    nc.tensor.matmul(pp[3][:], w1, xs1, **st)
                for i in range(4):
                    dst = dst0 if i < 2 else dst1
                    bc = bcol + (0 if i < 2 else 1)
                    qb = pr * 2 + (i % 2)
                    nc.vector.tensor_scalar(
                        dst[:, qb * QB: (qb + 1) * QB], pp[i][:],
                        bqk_sb[:, bc: bc + 1], None, ADD,
                    )

            def v_chunk(sc, halves, psV):
                # transposed v projection: pv[s, d] for S-chunk sc
                pv = psV.tile([P, DPG], F32, tag="pv", name="pv", bufs=2)
                for hc in range(NHC):
                    st = dict(start=(hc == 0), stop=(hc == NHC - 1))
                    xh = halves[hc // 4]
                    stat = xh[:, (hc % 4) * 1024 + (sc % 8) * P:
                              (hc % 4) * 1024 + (sc % 8) * P + P]
                    nc.tensor.matmul(
                        pv[:], stat, wv_sb[:, hc * DPG: (hc + 1) * DPG], **st
                    )
                # va65 blocks (sc*HPG+h)*65 : +64 <- pv[:, h*64 : +64] (*em +bv)
                dst = va65[:, sc * HPG * VA_W: (sc + 1) * HPG * VA_W]
                dst3 = dst.rearrange("p (h w) -> p h w", h=HPG, w=VA_W)[:, :, 0:HD]
                src3 = pv[:].rearrange("p (h w) -> p h w", h=HPG, w=HD)
                if bias_zero and mask_zero:
                    nc.vector.tensor_copy(dst3, src3)
                elif bias_zero:
                    nc.vector.tensor_scalar(
                        dst3, src3, em_sb[:, sc: sc + 1], None, MULT
                    )
                else:
                    bv3 = bvt_sb[:].rearrange(
                        "p (h w) -> p h w", h=HPG, w=HD
                    )
                    nc.vector.tensor_tensor(dst3, src3, bv3, ADD)
                    if not mask_zero:
                        nc.vector.tensor_scalar(
                            dst3, dst3, em_sb[:, sc: sc + 1], None, MULT
                        )

            # ---- phase P: q/k projections + early scores injection ----
            # DMA order tracks first-use order: the serial queue moves
            # ~0.35GB/s, so the first exp is gated on wk+xk+wq+xq (5MB).
            psB_box = [None]
            with tc.tile_pool(name="psP", bufs=1, space="PSUM") as psP:
                xlh = x_load_half
                k0h = [xlh(xk_d, 0, 0)]
                w_load(wk_sb, wk_d, 1)
                k0h.append(xlh(xk_d, 0, 1))
                w_load(wq_sb, wq_d, 0)
                w_load(wq_sb, wq_d, 1)
                q0h = [xlh(xq_d, 0, 0), xlh(xq_d, 0, 1)]
                bqk_sb = cpool.tile([P, 4], F32)
                nc.sync.dma_start(bqk_sb[:], bqk_d[:])
                em_sb = cpool.tile([P, NKC], F32)
                nc.sync.dma_start(em_sb[:], em_d[:])
                if not bias_zero:
                    bvt_sb = cpool.tile([P, DPG], F32)
                    nc.sync.dma_start(bvt_sb[:], bvt_d[:])
                # em columns of va65 (denominator source): row of em per kc
                emv = va65[:].rearrange(
                    "p (b w) -> p b w", b=NKC * HPG, w=VA_W
                )[:, :, HD: HD + 1]
                if mask_zero:
                    nc.vector.memset(emv, 1.0)
                else:
                    ones4 = cpool.tile([P, HPG], F32)
                    nc.vector.memset(ones4[:], 1.0)
                    for kc in range(NKC):
                        nc.vector.tensor_scalar(
                            emv[:, kc * HPG: (kc + 1) * HPG, :],
                            ones4[:].rearrange("p (h w) -> p h w", h=HPG, w=1),
                            em_sb[:, kc: kc + 1], None, MULT,
                        )
                hold = {}
                proj_pair(k0h, wk_sb, 2, kT0, kT1, 0, psP, False,
                          hook0=lambda: hold.update(a=xlh(xk_d, 1, 0)),
                          hook4=lambda: hold.update(b=xlh(xk_d, 1, 1)))
                proj_pair(q0h, wq_sb, 0, qT0, qT1, 0,
                          psP, False,
                          hook0=lambda: hold.update(c=xlh(xq_d, 1, 0)),
                          hook4=lambda: hold.update(d=xlh(xq_d, 1, 1)))
                state["ready"] = 32
                proj_pair([hold["a"], hold["b"]], wk_sb, 2, kT0, kT1, 1,
                          psP, True,
                          hook0=lambda: hold.update(a=xlh(xv_d, 0, 0)),
                          hook4=lambda: hold.update(b=xlh(xv_d, 0, 1)))
                state["ready"] = 64
                w_load(wv_sb, wv_d)
                proj_pair([hold["c"], hold["d"]], wq_sb, 0, qT0, qT1, 1,
                          psP, True)
                state["ready"] = NU

            # ---- v phase: transposed v projection + injection ----
            with tc.tile_pool(name="psB", bufs=1, space="PSUM") as psB:
                psB_box[0] = psB
                with tc.tile_pool(name="psV", bufs=1, space="PSUM") as psV:
                    vh = [hold["a"], hold["b"]]
                    for sc in range(NKC):
                        if sc == 2:
                            hold.update(a=xlh(xv_d, 1, 0))
                        if sc == 5:
                            hold.update(b=xlh(xv_d, 1, 1))
                        if sc == 8:
                            vh = [hold["a"], hold["b"]]
                        v_chunk(sc, vh, psV)
                        inject()
                        inject()

                # ---- phase 2: remaining scores/exp + interleaved ctx ----
                with tc.tile_pool(name="psC", bufs=1, space="PSUM") as psC:
                    cs = {"si": 0, "ki": 0}

                    def ctx_slot_ready(limit):
                        if cs["si"] >= 2 * NJ:
                            return False
                        j = cs["si"] // 2
                        return pos_of[(j, cs["ki"])] < limit

                    def emit_evac():
                        # deferred sweep evacuation: runs after the NEXT
                        # scores group's MMs so the DVE copy never blocks
                        # ScalarE's input matmuls.
                        X, h, qc, j, half = cs.pop("pe")
                        ou = oupool.tile([OU_H, QB], F32, tag="ou")
                        nc.vector.tensor_copy(ou[:], X[0:OU_H, :])
                        nc.sync.dma_start(
                            out_d[h * OU_H: (h + 1) * OU_H,
                                  qc * QB: (qc + 1) * QB],
                            ou[:],
                        )
                        if half == 1:
                            for kc in range(NKC):
                                pq[j * NKC + kc] = None

                    def emit_ctx_slot():
                        # returns True at a sweep boundary (evac left pending)
                        si, ki = cs["si"], cs["ki"]
                        j, half = si // 2, si % 2
                        qc, hp = divmod(j, 2)
                        h = hp * 2 + half
                        if ki == 0:
                            if "pe" in cs:
                                emit_evac()
                            cs["X"] = psC.tile(
                                [P, QB], F32, tag="X", name="X", bufs=2
                            )
                        X = cs["X"]
                        pt, off = pq[j * NKC + ki]
                        cons[pos_of[(j, ki)]] += 1
                        oh = off + half * 512
                        vo = (ki * HPG + h) * VA_W
                        st = dict(start=(ki == 0), stop=(ki == NKC - 1))
                        nc.tensor.matmul(
                            X[0:OU_H, :], va65[:, vo: vo + VA_W],
                            pt[:, oh: oh + QB], **st,
                        )
                        cs["ki"] += 1
                        if cs["ki"] == NKC:
                            cs["pe"] = (X, h, qc, j, half)
                            cs["si"] += 1
                            cs["ki"] = 0
                            return True
                        return False

                    def drain_ctx(budget, limit):
                        while budget > 0 and ctx_slot_ready(limit):
                            emit_ctx_slot()
                            budget -= 1

                    def force_drain():
                        assert ctx_slot_ready(NU)
                        emit_ctx_slot()

                    hooks["force_drain"] = force_drain

                    # one-time pre-drain under the cover of still-queued
                    # injection EXPs, so the first ring wraps don't burst
                    drain_ctx(10, state["u"])

                    while state["u"] < NU:
                        u_before = state["u"]
                        rem = NU - u_before
                        if rem >= 2:
                            scores_group("A2", psA)
                        else:
                            scores_group("A1", psA)
                        if state["u"] < NU:
                            scores_group("B1", psB)
                        n = state["u"] - u_before
                        drain_ctx(3 * n + 4, u_before)
                    while ctx_slot_ready(NU):
                        emit_ctx_slot()
                    if "pe" in cs:
                        emit_evac()
                    assert cs["si"] == 2 * NJ

    nc.compile()
    return nc


def _in_maps(query, key, value, attention_mask, Wq, bq, Wk, bk, Wv, bv):
    import ml_dtypes

    bf16 = ml_dtypes.bfloat16
    q = np.asarray(query, np.float32)
    k = np.asarray(key, np.float32)
    v = np.asarray(value, np.float32)
    m = np.asarray(attention_mask, np.float32)
    Wq = np.asarray(Wq, np.float32)
    Wk = np.asarray(Wk, np.float32)
    Wv = np.asarray(Wv, np.float32)
    bq = np.asarray(bq, np.float32)
    bk = np.asarray(bk, np.float32)
    bv = np.asarray(bv, np.float32)

    def prep_x(x):
        # [S, H] -> xT [H, S] -> pre-tiled [(pr hf p), (h c)] so each
        # x_load_half is one contiguous 2D DMA (see _build).
        xT = x.T  # [1024, 2048]
        return np.ascontiguousarray(
            xT.reshape(2, 4, P, 2, 1024).transpose(3, 0, 2, 1, 4)
            .reshape(4 * P, 4096)
        ).astype(bf16)

    def prep_w(W):
        # W slice [256, 1024] -> W^T [1024, 256] -> [p, (hc c)] SBUF layout
        WT = W.T  # [1024, 256]
        return np.ascontiguousarray(
            WT.reshape(NHC, P, DPG).transpose(1, 0, 2).reshape(P, NHC * DPG)
        ).astype(bf16)

    xT = [(prep_x(q[b]), prep_x(k[b]), prep_x(v[b])) for b in range(B)]
    maps = []
    for c in range(NCORES):
        b, hg = divmod(c, GROUPS)
        hs = hg * DPG
        he = hs + DPG
        bqs, bks, bvs = bq[hs:he], bk[hs:he], bv[hs:he]
        bqk = np.stack([bqs[:P], bqs[P:], bks[:P], bks[P:]], axis=1).astype(
            np.float32
        )
        bvt = np.broadcast_to(bvs, (P, DPG)).astype(np.float32)
        em = np.exp(m[b, 0, 0]).astype(np.float32)  # [S]
        maps.append(
            {
                "xq": xT[b][0],
                "xk": xT[b][1],
                "xv": xT[b][2],
                "wq": prep_w(Wq[hs:he, :]),
                "wk": prep_w(Wk[hs:he, :]),
                "wv": prep_w(Wv[hs:he, :]),
                "bqk": np.ascontiguousarray(bqk),
                "bvt": np.ascontiguousarray(bvt),
                "em": np.ascontiguousarray(em.reshape(NKC, P).T),
            }
        )
    return maps


def kernel(query, key, value, attention_mask, Wq, bq, Wk, bk, Wv, bv):
    from concourse.bass_utils import run_bass_kernel_spmd

    bias_zero = (
        not np.any(np.asarray(bq))
        and not np.any(np.asarray(bk))
        and not np.any(np.asarray(bv))
    )
    mask_zero = not np.any(np.asarray(attention_mask))
    nc = _build(bias_zero, mask_zero)
    maps = _in_maps(
        query, key, value, attention_mask, Wq, bq, Wk, bk, Wv, bv
    )
    res = run_bass_kernel_spmd(nc, maps, core_ids=list(range(NCORES)))
    out = np.empty((B, S, H), np.float32)
    for c in range(NCORES):
        b, hg = divmod(c, GROUPS)
        o = np.asarray(res.results[c]["out"], np.float32)  # [4*65, S]
        for h in range(HPG):
            num = o[h * OU_H: h * OU_H + HD]      # [64, S]
            den = o[h * OU_H + HD]                # [S]
            out[b, :, hg * DPG + h * HD: hg * DPG + (h + 1) * HD] = (num / den).T
    return out


# revision 19
# speedup vs baseline: 1.0139x; 1.0139x over previous
"""Multi-head attention (B=2, S=2048, H=1024, NH=16) on 8 TRN2 NeuronCores.

Sharding: fully data/tensor parallel, no collectives. Core c = (b, hg) with
b = c // 4 (batch), hg = c % 4 (head group of 4 heads = 256 of the 1024
projection output dims). Each core projects q/k/v for its 4 heads and runs
flash-style attention fully on-chip.

v4 design (from v3 trace analysis: PE busy 161us was the wall, ACT 131us):
  - 16 warmup matmuls on a memset tile at t=0 release the HAM clock gate
    (PE runs at 1.2GHz until ~3.4us of sustained activity) while input DMAs
    land, so projections start at 2.4GHz.
  - q/k projections use 1024-wide moving operands (2 PSUM-bank pp tiles,
    2 matmuls per contraction chunk) - halves matmul count vs v3, same
    cycles, less per-instruction latency exposure.
  - v projection emits v^T directly (stationary = x chunk, moving = Wv), so
    the 32 PE transposes of v3 are gone and va65 is built straight from the
    projection PSUM by one strided DVE copy per S-chunk.
  - ctx: ONE matmul per (sweep, kc) with full 128-row contraction and a
    [128, 65] va stationary (64 v dims + em column -> row 64 = softmax
    denominator). v3's X/Y row-split was pointless: same cycles, but 2x the
    PSUM banks, 2x the matmuls and an extra DVE add per sweep. Single
    [65, 512] fp32 accumulator per sweep = 1 PSUM bank, copied once to SBUF
    and DMA'd out as fp32 (host divides + transposes; fp32 out removes the
    bf16 rounding of num/den of v3).
  - ScalarE does nothing but exp: A2 groups ([128,2048], 4 banks) and B1
    groups ([128,1024], 2 banks). NDVE of the B1 groups can be moved to the
    Vector engine via a one-instruction Schraudolph exp (tensor_scalar
    fp32->int16 whose integer result IS the bf16 bit pattern of exp), to
    rebalance when ACT is the wall. Default 0.
  - PSUM budget: phase P = pp0+pp1 (4 banks) + A2 inject region (4);
    v phase = pv (1) + A2 (4) + B1 (2); phase 2 = A2(4)+B1(2)+ctx(2x1).
"""

import functools
import sys

if "/opt/trn_rl_repo" not in sys.path:
    sys.path.insert(0, "/opt/trn_rl_repo")

import numpy as np

B, S, H = 2, 2048, 1024
NH, HD = 16, 64
NCORES = 8
GROUPS = 4                # head groups (cores per batch)
DPG = H // GROUPS         # projection dims per core = 256
HPG = DPG // HD           # heads per core = 4
P = 128                   # SBUF partitions
NHC = H // P              # contraction chunks per projection = 8
QB = 512                  # q block
NQB = S // QB             # 4
NKC = S // P              # k chunks = 16
VA_W = HD + 1             # va block width per (kc, h) = 65
OU_H = HD + 1             # 65 output rows per sweep (64 dims + denom)
NJ = NQB * 2              # 8 (qc, hp) sweeps
NU = NJ * NKC             # 128 scores units
NDVE = 16                 # B1 units exp'd on VectorE (Schraudolph)

# Schraudolph exp for the DVE path: int16(round(s*EXPA + EXPB)) bit-viewed
# as bf16 equals exp(0.125*s) within ~3.3% (C=5.5 centers the sawtooth).
EXPA = 0.125 * 128.0 / float(np.log(2.0))
EXPB = 127.0 * 128.0 - 5.5


@functools.lru_cache(maxsize=2)
def _build(bias_zero=True, mask_zero=True, ndve=NDVE):
    import concourse.bacc as bacc
    import concourse.mybir as mybir
    import concourse.tile as tile

    F32 = mybir.dt.float32
    BF16 = mybir.dt.bfloat16
    I16 = mybir.dt.int16
    Exp = mybir.ActivationFunctionType.Exp
    MULT = mybir.AluOpType.mult
    ADD = mybir.AluOpType.add

    nc = bacc.Bacc()

    # x tensors pre-tiled on host: row (pr*2+hf)*128+p, col h*1024+c holds
    # xT[hf*512+h*128+p, pr*1024+c] -> each x_load is a contiguous 2D DMA
    # (8KB per partition line, minimal descriptor count).
    xq_d = nc.declare_dram_parameter("xq", [4 * P, 4096], BF16, isOutput=False)
    xk_d = nc.declare_dram_parameter("xk", [4 * P, 4096], BF16, isOutput=False)
    xv_d = nc.declare_dram_parameter("xv", [4 * P, 4096], BF16, isOutput=False)
    wq_d = nc.declare_dram_parameter("wq", [P, NHC * DPG], BF16, isOutput=False)
    wk_d = nc.declare_dram_parameter("wk", [P, NHC * DPG], BF16, isOutput=False)
    wv_d = nc.declare_dram_parameter("wv", [P, NHC * DPG], BF16, isOutput=False)
    bqk_d = nc.declare_dram_parameter("bqk", [P, 4], F32, isOutput=False)
    bvt_d = nc.declare_dram_parameter("bvt", [P, DPG], F32, isOutput=False)
    em_d = nc.declare_dram_parameter("em", [P, NKC], F32, isOutput=False)
    out_d = nc.declare_dram_parameter("out", [HPG * OU_H, S], F32, isOutput=True)

    # stream order: j = qc*2 + hp. Segmented by projection readiness:
    # seg1 (j0-3, kc0-7) needs only kT(kc0-7)+qT(qc0,1).
    units = (
        [(j, kc) for j in range(4) for kc in range(8)]
        + [(j, kc) for j in range(4) for kc in range(8, 16)]
        + [(j, kc) for j in range(4, NJ) for kc in range(NKC)]
    )
    pos_of = {jk: i for i, jk in enumerate(units)}
    assert len(units) == NU

    PA_BUFS = 12
    PB_BUFS = 24

    with tile.TileContext(nc) as tc:
        with (
            tc.tile_pool(name="const", bufs=1) as cpool,
            tc.tile_pool(name="proj_out", bufs=1) as projpool,
            tc.tile_pool(name="xt", bufs=6) as xpool,
            tc.tile_pool(name="pA", bufs=PA_BUFS) as ppA,
            tc.tile_pool(name="pB", bufs=PB_BUFS) as ppB,
            tc.tile_pool(name="ou", bufs=4) as oupool,
            tc.tile_pool(name="psA", bufs=1, space="PSUM") as psA,
        ):
            # ---- warmup: release the PE HAM clock gate while DMAs land,
            # and preload the exp ACT table so the first real exp is cheap ----
            warm_sb = cpool.tile([P, DPG], BF16)
            nc.gpsimd.memset(warm_sb[:], 0.0)
            warm_e = cpool.tile([P, 1], BF16)
            nc.scalar.activation(warm_e[:], warm_sb[:, 0:1], Exp)
            with tc.tile_pool(name="psW", bufs=1, space="PSUM") as psW:
                wps = psW.tile([P, DPG], F32, tag="warm", name="warm")
                for _ in range(22):
                    nc.tensor.matmul(
                        wps[:], warm_sb[:, 0:P], warm_sb[:], start=True, stop=True
                    )

            # ---- constants; split w loads so proj hc0 can start as soon as
            # the first half of wk + xk lands (DMA is ~0.35GB/s serial) ----
            wq_sb = cpool.tile([P, NHC * DPG], BF16)
            wk_sb = cpool.tile([P, NHC * DPG], BF16)
            wv_sb = cpool.tile([P, NHC * DPG], BF16)

            def w_load(w_sb, w_d, hf=None):
                if hf is None:
                    nc.sync.dma_start(w_sb[:], w_d[:])
                    return
                nc.sync.dma_start(
                    w_sb[:, hf * 1024: (hf + 1) * 1024],
                    w_d[:, hf * 1024: (hf + 1) * 1024],
                )

            w_load(wk_sb, wk_d, 0)

            # ---- persistent projection outputs ----
            qT0 = projpool.tile([P, S], BF16)   # heads 0,1 (hp=0)
            qT1 = projpool.tile([P, S], BF16)   # heads 2,3 (hp=1)
            kT0 = projpool.tile([P, S], BF16)
            kT1 = projpool.tile([P, S], BF16)
            va65 = projpool.tile([P, NKC * HPG * VA_W], BF16)
            nc.gpsimd.memset(va65[:], 0.0)

            qT = (qT0, qT1)
            kT = (kT0, kT1)

            # ---- scores/exp stream machinery ----
            pq = [None] * NU          # (j*NKC+kc) -> (p_tile, col offset)
            state = {"u": 0, "ready": 0, "a2inj": 0, "b1inj": 0, "injc": 0,
                     "b1n": 0}
            cons = [0] * NU           # ctx slots emitted per unit (2 = done)
            from collections import deque
            ring_hist = {"pA": deque(), "pB": deque(), "pA1": deque()}
            ring_bufs = {"pA": PA_BUFS, "pB": PB_BUFS, "pA1": 2}
            hooks = {"force_drain": None}

            def ensure_ring(tag, new_units):
                # before reusing a ring slot, its previous tile's ctx
                # consumers must have been emitted (else silent corruption)
                dq = ring_hist[tag]
                if len(dq) >= ring_bufs[tag]:
                    old = dq.popleft()
                    while any(cons[p] < 2 for p in old):
                        assert hooks["force_drain"] is not None, \
                            "ring wrap before ctx drain exists"
                        hooks["force_drain"]()
                dq.append(new_units)

            def scores_group(kind, psum_pool):
                n = 2 if kind == "A2" else 1
                u0 = state["u"]
                fd = 1024 * n
                if kind == "B1":
                    s = psum_pool.tile([P, 1024], F32, tag="sB", name="sB")
                else:
                    s = psum_pool.tile([P, 2048], F32, tag="sA", name="sA")
                for i in range(n):
                    j, kc = units[u0 + i]
                    qc, hp = divmod(j, 2)
                    for half in range(2):
                        rows = slice(half * 64, half * 64 + 64)
                        nc.tensor.matmul(
                            s[:, i * 1024 + half * 512: i * 1024 + (half + 1) * 512],
                            kT[hp][rows, kc * P: (kc + 1) * P],
                            qT[hp][rows, qc * QB: (qc + 1) * QB],
                            start=True,
                            stop=True,
                        )
                # p-tile allocation after the MMs: the ring guard's forced
                # ctx drains then cannot delay this group's score matmuls
                ensure_ring(
                    {"B1": "pB", "A2": "pA", "A1": "pA1"}[kind],
                    [u0 + i for i in range(n)],
                )
                if kind == "B1":
                    pt = ppB.tile([P, 1024], BF16, tag="pB", name="pB")
                elif kind == "A2":
                    pt = ppA.tile([P, 2048], BF16, tag="pA", name="pA")
                else:
                    pt = ppA.tile([P, 1024], BF16, tag="pA1", name="pA1", bufs=2)
                # B1 exp offload to VectorE: only in the main-loop era (the
                # injected groups must stay on ScalarE to cover its head, and
                # early DVE-exp bursts would starve the va65 build), and
                # alternating so the DVE also keeps up with ctx evacs.
                state["b1c"] = state.get("b1c", 0) + 1
                if (kind == "B1" and state["b1n"] < ndve and u0 >= 64
                        and state["b1c"] % 2 == 0):
                    state["b1n"] += 1
                    nc.vector.tensor_scalar(
                        pt[:, :fd].bitcast(I16), s[:, :fd], EXPA, EXPB, MULT, ADD
                    )
                else:
                    nc.scalar.activation(pt[:, :fd], s[:, :fd], Exp, scale=0.125)
                for i in range(n):
                    j, kc = units[u0 + i]
                    pq[j * NKC + kc] = (pt, i * 1024)
                state["u"] = u0 + n

            # ---- projection machinery ----
            def x_load_half(x_d, pr, hf):
                # half a pair (4 H-chunks x 1024 q-cols): contiguous 2D DMA
                xt = xpool.tile([P, 4096], BF16, tag="xt")
                r0 = (pr * 2 + hf) * P
                nc.sync.dma_start(xt[:], x_d[r0: r0 + P, :])
                return xt

            def inject():
                state["injc"] += 1
                if state["injc"] % 3 == 1:
                    return
                rem = state["ready"] - state["u"]
                if rem >= 2 and state["a2inj"] < 10:
                    state["a2inj"] += 1
                    scores_group("A2", psA)
                elif rem >= 1 and state["b1inj"] < 16 and psB_box[0] is not None:
                    state["b1inj"] += 1
                    scores_group("B1", psB_box[0])

            def proj_pair(halves, w_sb, bcol, dst0, dst1, pr, psP, inj,
                          hook0=None, hook4=None):
                # matmul output must fit one PSUM bank -> 4 x [128, 512].
                # hc0-3 run hc-major (matching DMA arrival of the two x
                # halves); hc4-7 run pp-major with the evac issued the moment
                # each pp stops, so the next pass's pp ring slot frees early
                # and the PE never idles (idle >3.4us re-throttles the clock).
                pp = [
                    psP.tile([P, QB], F32, tag=f"pp{i}", name=f"pp{i}", bufs=1)
                    for i in range(4)
                ]

                def mm(i, hc, **st):
                    xh = halves[hc // 4]
                    base = (hc % 4) * 1024
                    xs = xh[:, base + (i % 2) * QB: base + (i % 2) * QB + QB]
                    w = w_sb[:, hc * DPG + (i // 2) * P:
                             hc * DPG + (i // 2) * P + P]
                    nc.tensor.matmul(pp[i][:], w, xs, **st)

                def evac(i):
                    dst = dst0 if i < 2 else dst1
                    bc = bcol + (0 if i < 2 else 1)
                    qb = pr * 2 + (i % 2)
                    nc.vector.tensor_scalar(
                        dst[:, qb * QB: (qb + 1) * QB], pp[i][:],
                        bqk_sb[:, bc: bc + 1], None, ADD,
                    )

                for hc in range(4):
                    if hc == 0 and hook0 is not None:
                        hook0()
                    if inj:
                        inject()
                    for i in range(4):
                        mm(i, hc, start=(hc == 0), stop=False)
                if hook4 is not None:
                    hook4()
                for i in range(4):
                    if inj:
                        inject()
                    for hc in range(4, NHC):
                        mm(i, hc, start=False, stop=(hc == NHC - 1))
                    evac(i)

            def v_chunk(sc, halves, psV):
                # transposed v projection: pv[s, d] for S-chunk sc
                pv = psV.tile([P, DPG], F32, tag="pv", name="pv", bufs=2)
                for hc in range(NHC):
                    st = dict(start=(hc == 0), stop=(hc == NHC - 1))
                    xh = halves[hc // 4]
                    stat = xh[:, (hc % 4) * 1024 + (sc % 8) * P:
                              (hc % 4) * 1024 + (sc % 8) * P + P]
                    nc.tensor.matmul(
                        pv[:], stat, wv_sb[:, hc * DPG: (hc + 1) * DPG], **st
                    )
                # va65 blocks (sc*HPG+h)*65 : +64 <- pv[:, h*64 : +64] (*em +bv)
                dst = va65[:, sc * HPG * VA_W: (sc + 1) * HPG * VA_W]
                dst3 = dst.rearrange("p (h w) -> p h w", h=HPG, w=VA_W)[:, :, 0:HD]
                src3 = pv[:].rearrange("p (h w) -> p h w", h=HPG, w=HD)
                if bias_zero and mask_zero:
                    nc.vector.tensor_copy(dst3, src3)
                elif bias_zero:
                    nc.vector.tensor_scalar(
                        dst3, src3, em_sb[:, sc: sc + 1], None, MULT
                    )
                else:
                    bv3 = bvt_sb[:].rearrange(
                        "p (h w) -> p h w", h=HPG, w=HD
                    )
                    nc.vector.tensor_tensor(dst3, src3, bv3, ADD)
                    if not mask_zero:
                        nc.vector.tensor_scalar(
                            dst3, dst3, em_sb[:, sc: sc + 1], None, MULT
                        )

            # ---- phase P: q/k projections + early scores injection ----
            # DMA order tracks first-use order: the serial queue moves
            # ~0.35GB/s, so the first exp is gated on wk+xk+wq+xq (5MB).
            psB_box = [None]
            with tc.tile_pool(name="psP", bufs=1, space="PSUM") as psP:
                xlh = x_load_half
                k0h = [xlh(xk_d, 0, 0)]
                w_load(wk_sb, wk_d, 1)
                k0h.append(xlh(xk_d, 0, 1))
                w_load(wq_sb, wq_d, 0)
                w_load(wq_sb, wq_d, 1)
                q0h = [xlh(xq_d, 0, 0), xlh(xq_d, 0, 1)]
                bqk_sb = cpool.tile([P, 4], F32)
                nc.sync.dma_start(bqk_sb[:], bqk_d[:])
                em_sb = cpool.tile([P, NKC], F32)
                nc.sync.dma_start(em_sb[:], em_d[:])
                if not bias_zero:
                    bvt_sb = cpool.tile([P, DPG], F32)
                    nc.sync.dma_start(bvt_sb[:], bvt_d[:])
                # em columns of va65 (denominator source): row of em per kc
                emv = va65[:].rearrange(
                    "p (b w) -> p b w", b=NKC * HPG, w=VA_W
                )[:, :, HD: HD + 1]
                if mask_zero:
                    nc.vector.memset(emv, 1.0)
                else:
                    ones4 = cpool.tile([P, HPG], F32)
                    nc.vector.memset(ones4[:], 1.0)
                    for kc in range(NKC):
                        nc.vector.tensor_scalar(
                            emv[:, kc * HPG: (kc + 1) * HPG, :],
                            ones4[:].rearrange("p (h w) -> p h w", h=HPG, w=1),
                            em_sb[:, kc: kc + 1], None, MULT,
                        )
                hold = {}
                proj_pair(k0h, wk_sb, 2, kT0, kT1, 0, psP, False,
                          hook0=lambda: hold.update(a=xlh(xk_d, 1, 0)),
                          hook4=lambda: hold.update(b=xlh(xk_d, 1, 1)))
                proj_pair(q0h, wq_sb, 0, qT0, qT1, 0,
                          psP, False,
                          hook0=lambda: hold.update(c=xlh(xq_d, 1, 0)),
                          hook4=lambda: hold.update(d=xlh(xq_d, 1, 1)))
                state["ready"] = 32
                proj_pair([hold["a"], hold["b"]], wk_sb, 2, kT0, kT1, 1,
                          psP, True,
                          hook0=lambda: hold.update(a=xlh(xv_d, 0, 0)),
                          hook4=lambda: hold.update(b=xlh(xv_d, 0, 1)))
                state["ready"] = 64
                w_load(wv_sb, wv_d)
                proj_pair([hold["c"], hold["d"]], wq_sb, 0, qT0, qT1, 1,
                          psP, True)
                state["ready"] = NU

            # ---- v phase: transposed v projection + injection ----
            with tc.tile_pool(name="psB", bufs=1, space="PSUM") as psB:
                psB_box[0] = psB
                with tc.tile_pool(name="psV", bufs=1, space="PSUM") as psV:
                    vh = [hold["a"], hold["b"]]
                    for sc in range(NKC):
                        if sc == 2:
                            hold.update(a=xlh(xv_d, 1, 0))
                        if sc == 5:
                            hold.update(b=xlh(xv_d, 1, 1))
                        if sc == 8:
                            vh = [hold["a"], hold["b"]]
                        v_chunk(sc, vh, psV)
                        inject()
                        inject()

                # ---- phase 2: remaining scores/exp + interleaved ctx ----
                with tc.tile_pool(name="psC", bufs=1, space="PSUM") as psC:
                    cs = {"si": 0, "ki": 0}

                    def ctx_slot_ready(limit):
                        if cs["si"] >= 2 * NJ:
                            return False
                        j = cs["si"] // 2
                        return pos_of[(j, cs["ki"])] < limit

                    def emit_evac():
                        # deferred sweep evacuation: runs after the NEXT
                        # scores group's MMs so the DVE copy never blocks
                        # ScalarE's input matmuls.
                        X, h, qc, j, half = cs.pop("pe")
                        ou = oupool.tile([OU_H, QB], F32, tag="ou")
                        nc.vector.tensor_copy(ou[:], X[0:OU_H, :])
                        nc.sync.dma_start(
                            out_d[h * OU_H: (h + 1) * OU_H,
                                  qc * QB: (qc + 1) * QB],
                            ou[:],
                        )
                        if half == 1:
                            for kc in range(NKC):
                                pq[j * NKC + kc] = None

                    def emit_ctx_slot():
                        # returns True at a sweep boundary (evac left pending)
                        si, ki = cs["si"], cs["ki"]
                        j, half = si // 2, si % 2
                        qc, hp = divmod(j, 2)
                        h = hp * 2 + half
                        if ki == 0:
                            if "pe" in cs:
                                emit_evac()
                            cs["X"] = psC.tile(
                                [P, QB], F32, tag="X", name="X", bufs=2
                            )
                        X = cs["X"]
                        pt, off = pq[j * NKC + ki]
                        cons[pos_of[(j, ki)]] += 1
                        oh = off + half * 512
                        vo = (ki * HPG + h) * VA_W
                        st = dict(start=(ki == 0), stop=(ki == NKC - 1))
                        nc.tensor.matmul(
                            X[0:OU_H, :], va65[:, vo: vo + VA_W],
                            pt[:, oh: oh + QB], **st,
                        )
                        cs["ki"] += 1
                        if cs["ki"] == NKC:
                            cs["pe"] = (X, h, qc, j, half)
                            cs["si"] += 1
                            cs["ki"] = 0
                            return True
                        return False

                    def drain_ctx(budget, limit):
                        while budget > 0 and ctx_slot_ready(limit):
                            emit_ctx_slot()
                            budget -= 1

                    def force_drain():
                        assert ctx_slot_ready(NU)
                        emit_ctx_slot()

                    hooks["force_drain"] = force_drain

                    # one-time pre-drain under the cover of still-queued
                    # injection EXPs, so the first ring wraps don't burst
                    drain_ctx(10, state["u"])

                    while state["u"] < NU:
                        u_before = state["u"]
                        rem = NU - u_before
                        if rem >= 2:
                            scores_group("A2", psA)
                        else:
                            scores_group("A1", psA)
                        if state["u"] < NU:
                            scores_group("B1", psB)
                        n = state["u"] - u_before
                        drain_ctx(3 * n + 4, u_before)
                    while ctx_slot_ready(NU):
                        emit_ctx_slot()
                    if "pe" in cs:
                        emit_evac()
                    assert cs["si"] == 2 * NJ

    nc.compile()
    return nc


def _in_maps(query, key, value, attention_mask, Wq, bq, Wk, bk, Wv, bv):
    import ml_dtypes

    bf16 = ml_dtypes.bfloat16
    q = np.asarray(query, np.float32)
    k = np.asarray(key, np.float32)
    v = np.asarray(value, np.float32)
    m = np.asarray(attention_mask, np.float32)
    Wq = np.asarray(Wq, np.float32)
    Wk = np.asarray(Wk, np.float32)
    Wv = np.asarray(Wv, np.float32)
    bq = np.asarray(bq, np.float32)
    bk = np.asarray(bk, np.float32)
    bv = np.asarray(bv, np.float32)

    def prep_x(x):
        # [S, H] -> xT [H, S] -> pre-tiled [(pr hf p), (h c)] so each
        # x_load_half is one contiguous 2D DMA (see _build).
        xT = x.T  # [1024, 2048]
        return np.ascontiguousarray(
            xT.reshape(2, 4, P, 2, 1024).transpose(3, 0, 2, 1, 4)
            .reshape(4 * P, 4096)
        ).astype(bf16)

    def prep_w(W):
        # W slice [256, 1024] -> W^T [1024, 256] -> [p, (hc c)] SBUF layout
        WT = W.T  # [1024, 256]
        return np.ascontiguousarray(
            WT.reshape(NHC, P, DPG).transpose(1, 0, 2).reshape(P, NHC * DPG)
        ).astype(bf16)

    xT = [(prep_x(q[b]), prep_x(k[b]), prep_x(v[b])) for b in range(B)]
    maps = []
    for c in range(NCORES):
        b, hg = divmod(c, GROUPS)
        hs = hg * DPG
        he = hs + DPG
        bqs, bks, bvs = bq[hs:he], bk[hs:he], bv[hs:he]
        bqk = np.stack([bqs[:P], bqs[P:], bks[:P], bks[P:]], axis=1).astype(
            np.float32
        )
        bvt = np.broadcast_to(bvs, (P, DPG)).astype(np.float32)
        em = np.exp(m[b, 0, 0]).astype(np.float32)  # [S]
        maps.append(
            {
                "xq": xT[b][0],
                "xk": xT[b][1],
                "xv": xT[b][2],
                "wq": prep_w(Wq[hs:he, :]),
                "wk": prep_w(Wk[hs:he, :]),
                "wv": prep_w(Wv[hs:he, :]),
                "bqk": np.ascontiguousarray(bqk),
                "bvt": np.ascontiguousarray(bvt),
                "em": np.ascontiguousarray(em.reshape(NKC, P).T),
            }
        )
    return maps


def kernel(query, key, value, attention_mask, Wq, bq, Wk, bk, Wv, bv):
    from concourse.bass_utils import run_bass_kernel_spmd

    bias_zero = (
        not np.any(np.asarray(bq))
        and not np.any(np.asarray(bk))
        and not np.any(np.asarray(bv))
    )
    mask_zero = not np.any(np.asarray(attention_mask))
    nc = _build(bias_zero, mask_zero)
    maps = _in_maps(
        query, key, value, attention_mask, Wq, bq, Wk, bk, Wv, bv
    )
    res = run_bass_kernel_spmd(nc, maps, core_ids=list(range(NCORES)))
    out = np.empty((B, S, H), np.float32)
    for c in range(NCORES):
        b, hg = divmod(c, GROUPS)
        o = np.asarray(res.results[c]["out"], np.float32)  # [4*65, S]
        for h in range(HPG):
            num = o[h * OU_H: h * OU_H + HD]      # [64, S]
            den = o[h * OU_H + HD]                # [S]
            out[b, :, hg * DPG + h * HD: hg * DPG + (h + 1) * HD] = (num / den).T
    return out
